# revision 4
# baseline (speedup 1.0000x reference)
"""QSP expectation kernel for Trainium2 (8 NeuronCores, data-parallel).

Math: the reference computes preds = alphas * Re(<0|U|0>) + bias where U is a
QSP chain with 55 phases. Re(<0|U|0>) as a function of theta is EXACTLY a
truncated Fourier series with period pi:

    f(theta) = c0 + sum_{k=1..27} A_k * sin(2k*theta + G_k)

(55 real degrees of freedom = 55 phases). The coefficients are recovered on
the host from 55 samples of the (cheap, 55-point) recurrence via FFT; the
spectrum decays exponentially in k, so only the first T (~14-20) harmonics
are needed for ~1e-3 relative accuracy.

Device kernel per core (65536 elements as [128, 512] f32), per harmonic k:
  1. DVE  tensor_scalar:  n = int32((theta + s1_k) * (1/m_k))   [trunc]
     with m_k = pi/k, s1_k = delta_k + D_k  (range reduction mod m_k; cast=RNE)
  2. DVE/GPSIMD stt:      r = (n * -m_k) + theta                 [residual]
  3. ACT  Sin:            h = sin(r * 2k + B_k),  arg in (-pi, pi]
  4. PE   diag-matmul:    acc(PSUM) += A_k * h    (lhsT = A_k * I, host-baked)
Epilogue: preds = (acc + c0) * alphas + bias  (DVE), DMA out.
"""

import numpy as np

import concourse.bass as bass
import concourse.tile as tile
from concourse import mybir as mb
from concourse.bass_utils import run_bass_kernel_spmd

QSP_DEPTH = 27
N_PHIS = 2 * QSP_DEPTH + 1  # 55
B = 524288
N_CORES = 8
P = 128
F = B // N_CORES // P  # 512
REL_TOL_TARGET = 1e-3  # truncation budget (harness gate is 2e-2)
T_MIN, T_MAX = 8, 27
GPS_FIXUP_EVERY = 10**9  # TensorScalarPtr is rejected on Pool by this walrus; keep fixups on DVE


# ---------------------------------------------------------------------------
# host-side math
# ---------------------------------------------------------------------------

def _qsp_scalar(theta, phis):
    """Reference recurrence evaluated in float64 on a small grid."""
    theta = np.asarray(theta, dtype=np.float64)
    phis = np.asarray(phis, dtype=np.float64)
    c, s = np.cos(theta), np.sin(theta)
    r0r = np.ones_like(theta)
    r0i = np.zeros_like(theta)
    r1r = np.zeros_like(theta)
    r1i = np.zeros_like(theta)
    for phi in phis[1:]:
        cp, sp = np.cos(phi), np.sin(phi)
        ar = r0r * c - r1i * s
        ai = r0i * c + r1r * s
        br = r1r * c - r0i * s
        bi = r1i * c + r0r * s
        r0r = ar * cp - ai * sp
        r0i = ar * sp + ai * cp
        r1r = br * cp + bi * sp
        r1i = bi * cp - br * sp
    return r0r * np.cos(phis[0]) - r0i * np.sin(phis[0])


def _fourier_coeffs(phis):
    """f(theta) = c0 + sum A_k sin(2k theta + G_k); exact for the QSP chain."""
    j = np.arange(N_PHIS)
    theta_j = np.pi * j / N_PHIS
    C = np.fft.fft(_qsp_scalar(theta_j, phis)) / N_PHIS
    c0 = float(np.real(C[0]))
    A = 2.0 * np.abs(C[1 : QSP_DEPTH + 1])
    G = np.angle(C[1 : QSP_DEPTH + 1]) + np.pi / 2
    return c0, A, G


def _choose_T(c0, A):
    meanp = c0 * c0 + float((A**2).sum()) / 2.0  # mean f^2 over the period
    meanp = max(meanp, 1e-12)
    for T in range(T_MIN, T_MAX):
        tail = float((A[T:] ** 2).sum()) / 2.0
        if np.sqrt(tail / meanp) < REL_TOL_TARGET:
            return T
    return T_MAX


# ---------------------------------------------------------------------------
# device program
# ---------------------------------------------------------------------------

def _split_excess_waits(nc):
    """This walrus build only supports ONE sem-wait per instruction; spill
    extra waits onto preceding same-engine NOPs."""
    cnt = 0
    for bb in nc.main_func.blocks:
        out, changed = [], False
        for ins in list(bb.instructions):
            si = ins.sync_info
            if si is not None and len(si.on_wait) > 1:
                waits = list(si.on_wait)
                for w in waits[1:]:
                    nop = mb.InstNoOp(name=f"waitsplit-{cnt}", ins=[], outs=[])
                    cnt += 1
                    nop.engine = ins.engine
                    nop.sync_info = mb.SyncInfo(on_wait=[w], on_update=[])
                    nc.register_instruction(nop)
                    out.append(nop)
                ins.sync_info = mb.SyncInfo(on_wait=waits[:1], on_update=list(si.on_update))
                changed = True
            out.append(ins)
        if changed:
            bb.instructions = out
    return cnt


_NC_CACHE = {}


def _build_nc(T):
    if T in _NC_CACHE:
        return _NC_CACHE[T]

    nc = bass.Bass("TRN2", target_bir_lowering=False, debug=False, num_devices=N_CORES)
    f32, i32 = mb.dt.float32, mb.dt.int32

    th_d = nc.dram_tensor("theta", [P, F], f32, kind="ExternalInput")
    al_d = nc.dram_tensor("alphas", [P, F], f32, kind="ExternalInput")
    par_d = nc.dram_tensor("params", [P, 3 * T + 2], f32, kind="ExternalInput")
    dg_d = nc.dram_tensor("diags", [P, T * P], f32, kind="ExternalInput")
    out_d = nc.dram_tensor("preds", [P, F], f32, kind="ExternalOutput")

    with tile.TileContext(nc) as tc:
        with (
            tc.tile_pool(name="persist", bufs=1) as persist,
            tc.tile_pool(name="work", bufs=4) as work,
            tc.tile_pool(name="psum", bufs=1, space="PSUM") as psum,
        ):
            th = persist.tile([P, F], f32, tag="th")
            nc.sync.dma_start(th[:], th_d.ap())
            al = persist.tile([P, F], f32, tag="al")
            nc.sync.dma_start(al[:], al_d.ap())
            par = persist.tile([P, 3 * T + 2], f32, tag="par")
            nc.sync.dma_start(par[:], par_d.ap())
            dg = persist.tile([P, T * P], f32, tag="dg")
            nc.sync.dma_start(dg[:], dg_d.ap())

            acc = psum.tile([P, F], f32, tag="acc")

            for i in range(T):
                k = i + 1
                m_k = float(np.float32(np.pi / k))
                n_t = work.tile([P, F], i32, tag="n")
                # n = int32((theta + s1) * (1/m))  -- trunc toward zero
                nc.vector.tensor_scalar(
                    n_t[:], th[:], par[:, 3 * i : 3 * i + 1], float(1.0 / (np.pi / k)),
                    mb.AluOpType.add, mb.AluOpType.mult,
                )
                r_t = work.tile([P, F], f32, tag="r")
                eng = nc.gpsimd if (i % GPS_FIXUP_EVERY == GPS_FIXUP_EVERY - 1) else nc.vector
                eng.scalar_tensor_tensor(
                    r_t[:], n_t[:], -m_k, th[:],
                    mb.AluOpType.mult, mb.AluOpType.add,
                )
                h_t = work.tile([P, F], f32, tag="h")
                nc.scalar.activation(
                    h_t[:], r_t[:], mb.ActivationFunctionType.Sin,
                    bias=par[:, 3 * i + 1 : 3 * i + 2], scale=float(2.0 * k),
                )
                nc.tensor.matmul(
                    acc[:], dg[:, i * P : (i + 1) * P], h_t[:],
                    start=(i == 0), stop=(i == T - 1),
                )

            # preds = (acc + c0) * alphas + bias
            tmp = work.tile([P, F], f32, tag="tmp")
            nc.vector.scalar_tensor_tensor(
                tmp[:], acc[:], par[:, 3 * T : 3 * T + 1], al[:],
                mb.AluOpType.add, mb.AluOpType.mult,
            )
            pred_t = work.tile([P, F], f32, tag="pred")
            nc.vector.tensor_scalar(
                pred_t[:], tmp[:], par[:, 3 * T + 1 : 3 * T + 2], None,
                mb.AluOpType.add,
            )
            nc.sync.dma_start(out_d.ap(), pred_t[:])

    _split_excess_waits(nc)
    _NC_CACHE[T] = nc
    return nc


# ---------------------------------------------------------------------------
# entry point
# ---------------------------------------------------------------------------

def kernel(x, qsp_params, alphas, bias):
    x = np.asarray(x, dtype=np.float32)
    qsp_params = np.asarray(qsp_params, dtype=np.float64)
    alphas = np.asarray(alphas, dtype=np.float32)
    bias_v = float(np.asarray(bias, dtype=np.float64).reshape(-1)[0])

    c0, A, G = _fourier_coeffs(qsp_params)
    T = _choose_T(c0, A)

    ks = np.arange(1, T + 1, dtype=np.float64)
    m = np.pi / ks
    D = np.ceil(8.0 / m) * m  # positive shift, integer multiple of m
    delta = np.mod(G[:T], 2 * np.pi) / (2 * ks)  # in [0, m)
    s1 = delta + D  # ts1 add-scalar (HW fp32->int32 cast rounds to nearest)
    Bact = 2 * ks * (delta + D)  # ACT bias: arg = 2k*r + Bact in (-pi, pi]

    params = np.zeros((P, 3 * T + 2), dtype=np.float32)
    for i in range(T):
        params[:, 3 * i] = s1[i]
        params[:, 3 * i + 1] = Bact[i]
        params[:, 3 * i + 2] = A[i]  # unused by device now (baked in diags)
    params[:, 3 * T] = c0
    params[:, 3 * T + 1] = bias_v

    eye = np.eye(P, dtype=np.float32)
    diags = np.concatenate([A[i] * eye for i in range(T)], axis=1).astype(np.float32)
    diags = np.ascontiguousarray(diags)

    theta = x[:, 0]
    per_core = B // N_CORES
    in_maps = []
    for c in range(N_CORES):
        sl = slice(c * per_core, (c + 1) * per_core)
        in_maps.append(
            {
                "theta": np.ascontiguousarray(theta[sl].reshape(P, F)),
                "alphas": np.ascontiguousarray(alphas[sl].reshape(P, F)),
                "params": params,
                "diags": diags,
            }
        )

    nc = _build_nc(T)
    res = run_bass_kernel_spmd(nc, in_maps, core_ids=list(range(N_CORES)))

    out = np.empty((B,), dtype=np.float32)
    for c in range(N_CORES):
        out[c * per_core : (c + 1) * per_core] = res.results[c]["preds"].reshape(-1)
    return out[:, None]


# revision 5
# speedup vs baseline: 1.0526x; 1.0526x over previous
"""QSP expectation kernel for Trainium2 (8 NeuronCores, data-parallel).

Math: the reference computes preds = alphas * Re(<0|U|0>) + bias where U is a
QSP chain with 55 phases. Re(<0|U|0>) as a function of theta is EXACTLY a
truncated Fourier series with period pi:

    f(theta) = c0 + sum_{k=1..27} A_k * sin(2k*theta + G_k)

(55 real degrees of freedom = 55 phases). The coefficients are recovered on
the host from 55 samples of the (cheap, 55-point) recurrence via FFT; the
spectrum decays exponentially in k, so only the first T (~14-20) harmonics
are needed for ~1e-3 relative accuracy.

Device kernel per core (65536 elements as [128, 512] f32), per harmonic k:
  1. DVE  tensor_scalar:  n = int32((theta + s1_k) * (1/m_k))   [trunc]
     with m_k = pi/k, s1_k = delta_k + D_k  (range reduction mod m_k; cast=RNE)
  2. DVE/GPSIMD stt:      r = (n * -m_k) + theta                 [residual]
  3. ACT  Sin:            h = sin(r * 2k + B_k),  arg in (-pi, pi]
  4. PE   diag-matmul:    acc(PSUM) += A_k * h    (lhsT = A_k * I, host-baked)
Epilogue: preds = (acc + c0) * alphas + bias  (DVE), DMA out.
"""

import numpy as np

import concourse.bass as bass
import concourse.tile as tile
from concourse import mybir as mb
from concourse.bass_utils import run_bass_kernel_spmd

QSP_DEPTH = 27
N_PHIS = 2 * QSP_DEPTH + 1  # 55
B = 524288
N_CORES = 8
P = 128
F = B // N_CORES // P  # 512
REL_TOL_TARGET = 1e-3  # truncation budget (harness gate is 2e-2)
T_MIN, T_MAX = 8, 27
GPS_FIXUP_EVERY = 10**9  # TensorScalarPtr is rejected on Pool by this walrus; keep fixups on DVE


# ---------------------------------------------------------------------------
# host-side math
# ---------------------------------------------------------------------------

def _qsp_scalar(theta, phis):
    """Reference recurrence evaluated in float64 on a small grid."""
    theta = np.asarray(theta, dtype=np.float64)
    phis = np.asarray(phis, dtype=np.float64)
    c, s = np.cos(theta), np.sin(theta)
    r0r = np.ones_like(theta)
    r0i = np.zeros_like(theta)
    r1r = np.zeros_like(theta)
    r1i = np.zeros_like(theta)
    for phi in phis[1:]:
        cp, sp = np.cos(phi), np.sin(phi)
        ar = r0r * c - r1i * s
        ai = r0i * c + r1r * s
        br = r1r * c - r0i * s
        bi = r1i * c + r0r * s
        r0r = ar * cp - ai * sp
        r0i = ar * sp + ai * cp
        r1r = br * cp + bi * sp
        r1i = bi * cp - br * sp
    return r0r * np.cos(phis[0]) - r0i * np.sin(phis[0])


def _fourier_coeffs(phis):
    """f(theta) = c0 + sum A_k sin(2k theta + G_k); exact for the QSP chain."""
    j = np.arange(N_PHIS)
    theta_j = np.pi * j / N_PHIS
    C = np.fft.fft(_qsp_scalar(theta_j, phis)) / N_PHIS
    c0 = float(np.real(C[0]))
    A = 2.0 * np.abs(C[1 : QSP_DEPTH + 1])
    G = np.angle(C[1 : QSP_DEPTH + 1]) + np.pi / 2
    return c0, A, G


def _choose_T(c0, A):
    meanp = c0 * c0 + float((A**2).sum()) / 2.0  # mean f^2 over the period
    meanp = max(meanp, 1e-12)
    for T in range(T_MIN, T_MAX):
        tail = float((A[T:] ** 2).sum()) / 2.0
        if np.sqrt(tail / meanp) < REL_TOL_TARGET:
            return T
    return T_MAX


# ---------------------------------------------------------------------------
# device program
# ---------------------------------------------------------------------------

def _split_excess_waits(nc):
    """This walrus build only supports ONE sem-wait per instruction; spill
    extra waits onto preceding same-engine NOPs."""
    cnt = 0
    for bb in nc.main_func.blocks:
        out, changed = [], False
        for ins in list(bb.instructions):
            si = ins.sync_info
            if si is not None and len(si.on_wait) > 1:
                waits = list(si.on_wait)
                for w in waits[1:]:
                    nop = mb.InstNoOp(name=f"waitsplit-{cnt}", ins=[], outs=[])
                    cnt += 1
                    nop.engine = ins.engine
                    nop.sync_info = mb.SyncInfo(on_wait=[w], on_update=[])
                    nc.register_instruction(nop)
                    out.append(nop)
                ins.sync_info = mb.SyncInfo(on_wait=waits[:1], on_update=list(si.on_update))
                changed = True
            out.append(ins)
        if changed:
            bb.instructions = out
    return cnt


_NC_CACHE = {}


def _build_nc(T):
    if T in _NC_CACHE:
        return _NC_CACHE[T]

    nc = bass.Bass("TRN2", target_bir_lowering=False, debug=False, num_devices=N_CORES)
    f32, i32 = mb.dt.float32, mb.dt.int32

    th_d = nc.dram_tensor("theta", [P, F], f32, kind="ExternalInput")
    al_d = nc.dram_tensor("alphas", [P, F], f32, kind="ExternalInput")
    par_d = nc.dram_tensor("params", [P, 3 * T + 2], f32, kind="ExternalInput")
    dg_d = nc.dram_tensor("diags", [P, T * P], mb.dt.bfloat16, kind="ExternalInput")
    out_d = nc.dram_tensor("preds", [P, F], f32, kind="ExternalOutput")

    with tile.TileContext(nc) as tc:
        with (
            tc.tile_pool(name="persist", bufs=1) as persist,
            tc.tile_pool(name="work", bufs=6) as work,
            tc.tile_pool(name="psum", bufs=1, space="PSUM") as psum,
        ):
            th = persist.tile([P, F], f32, tag="th")
            nc.sync.dma_start(th[:], th_d.ap())
            al = persist.tile([P, F], f32, tag="al")
            nc.sync.dma_start(al[:], al_d.ap())
            par = persist.tile([P, 3 * T + 2], f32, tag="par")
            nc.sync.dma_start(par[:], par_d.ap())
            dg = persist.tile([P, T * P], mb.dt.bfloat16, tag="dg")
            nc.sync.dma_start(dg[:], dg_d.ap())

            acc = psum.tile([P, F], f32, tag="acc")

            for i in range(T):
                k = i + 1
                m_k = float(np.float32(np.pi / k))
                n_t = work.tile([P, F], i32, tag="n")
                # n = int32((theta + s1) * (1/m))  -- trunc toward zero
                nc.vector.tensor_scalar(
                    n_t[:], th[:], par[:, 3 * i : 3 * i + 1], float(1.0 / (np.pi / k)),
                    mb.AluOpType.add, mb.AluOpType.mult,
                )
                r_t = work.tile([P, F], f32, tag="r")
                eng = nc.gpsimd if (i % GPS_FIXUP_EVERY == GPS_FIXUP_EVERY - 1) else nc.vector
                eng.scalar_tensor_tensor(
                    r_t[:], n_t[:], -m_k, th[:],
                    mb.AluOpType.mult, mb.AluOpType.add,
                )
                h_t = work.tile([P, F], mb.dt.bfloat16, tag="h")
                nc.scalar.activation(
                    h_t[:], r_t[:], mb.ActivationFunctionType.Sin,
                    bias=par[:, 3 * i + 1 : 3 * i + 2], scale=float(2.0 * k),
                )
                nc.tensor.matmul(
                    acc[:], dg[:, i * P : (i + 1) * P], h_t[:],
                    start=(i == 0), stop=(i == T - 1),
                )

            # preds = (acc + c0) * alphas + bias
            tmp = work.tile([P, F], f32, tag="tmp")
            nc.vector.scalar_tensor_tensor(
                tmp[:], acc[:], par[:, 3 * T : 3 * T + 1], al[:],
                mb.AluOpType.add, mb.AluOpType.mult,
            )
            pred_t = work.tile([P, F], f32, tag="pred")
            nc.vector.tensor_scalar(
                pred_t[:], tmp[:], par[:, 3 * T + 1 : 3 * T + 2], None,
                mb.AluOpType.add,
            )
            nc.sync.dma_start(out_d.ap(), pred_t[:])

    _split_excess_waits(nc)
    _NC_CACHE[T] = nc
    return nc


# ---------------------------------------------------------------------------
# entry point
# ---------------------------------------------------------------------------

def kernel(x, qsp_params, alphas, bias):
    x = np.asarray(x, dtype=np.float32)
    qsp_params = np.asarray(qsp_params, dtype=np.float64)
    alphas = np.asarray(alphas, dtype=np.float32)
    bias_v = float(np.asarray(bias, dtype=np.float64).reshape(-1)[0])

    c0, A, G = _fourier_coeffs(qsp_params)
    T = _choose_T(c0, A)

    ks = np.arange(1, T + 1, dtype=np.float64)
    m = np.pi / ks
    D = np.ceil(8.0 / m) * m  # positive shift, integer multiple of m
    delta = np.mod(G[:T], 2 * np.pi) / (2 * ks)  # in [0, m)
    s1 = delta + D  # ts1 add-scalar (HW fp32->int32 cast rounds to nearest)
    Bact = 2 * ks * (delta + D)  # ACT bias: arg = 2k*r + Bact in (-pi, pi]

    params = np.zeros((P, 3 * T + 2), dtype=np.float32)
    for i in range(T):
        params[:, 3 * i] = s1[i]
        params[:, 3 * i + 1] = Bact[i]
        params[:, 3 * i + 2] = A[i]  # unused by device now (baked in diags)
    params[:, 3 * T] = c0
    params[:, 3 * T + 1] = bias_v

    eye = np.eye(P, dtype=np.float32)
    import ml_dtypes
    diags = np.concatenate([A[i] * eye for i in range(T)], axis=1).astype(ml_dtypes.bfloat16)
    diags = np.ascontiguousarray(diags)

    theta = x[:, 0]
    per_core = B // N_CORES
    in_maps = []
    for c in range(N_CORES):
        sl = slice(c * per_core, (c + 1) * per_core)
        in_maps.append(
            {
                "theta": np.ascontiguousarray(theta[sl].reshape(P, F)),
                "alphas": np.ascontiguousarray(alphas[sl].reshape(P, F)),
                "params": params,
                "diags": diags,
            }
        )

    nc = _build_nc(T)
    res = run_bass_kernel_spmd(nc, in_maps, core_ids=list(range(N_CORES)))

    out = np.empty((B,), dtype=np.float32)
    for c in range(N_CORES):
        out[c * per_core : (c + 1) * per_core] = res.results[c]["preds"].reshape(-1)
    return out[:, None]


# revision 6
# speedup vs baseline: 1.1370x; 1.0802x over previous
"""QSP expectation kernel for Trainium2 (8 NeuronCores, data-parallel).

Math: the reference computes preds = alphas * Re(<0|U|0>) + bias where U is a
QSP chain with 55 phases. Re(<0|U|0>) as a function of theta is EXACTLY a
truncated Fourier series with period pi:

    f(theta) = c0 + sum_{k=1..27} A_k * sin(2k*theta + G_k)

(55 real degrees of freedom = 55 phases). The coefficients are recovered on
the host from 55 samples of the (cheap, 55-point) recurrence via FFT; the
spectrum decays exponentially in k, so only the first T (~14-20) harmonics
are needed for ~1e-3 relative accuracy.

Device kernel per core (65536 elements as [128, 512] f32), per harmonic k:
  1. DVE  tensor_scalar:  n = int32((theta + s1_k) * (1/m_k))   [trunc]
     with m_k = pi/k, s1_k = delta_k + D_k  (range reduction mod m_k; cast=RNE)
  2. DVE/GPSIMD stt:      r = (n * -m_k) + theta                 [residual]
  3. ACT  Sin:            h = sin(r * 2k + B_k),  arg in (-pi, pi]
  4. PE   diag-matmul:    acc(PSUM) += A_k * h    (lhsT = A_k * I, host-baked)
Epilogue: preds = (acc + c0) * alphas + bias  (DVE), DMA out.
"""

import numpy as np

import concourse.bass as bass
import concourse.tile as tile
from concourse import mybir as mb
from concourse.bass_utils import run_bass_kernel_spmd

QSP_DEPTH = 27
N_PHIS = 2 * QSP_DEPTH + 1  # 55
B = 524288
N_CORES = 8
P = 128
F = B // N_CORES // P  # 512
REL_TOL_TARGET = 3e-3  # truncation budget (harness gate is 2e-2; bf16 adds ~1.5e-3)
T_MIN, T_MAX = 8, 27
GPS_FIXUP_EVERY = 10**9  # TensorScalarPtr is rejected on Pool by this walrus; keep fixups on DVE


# ---------------------------------------------------------------------------
# host-side math
# ---------------------------------------------------------------------------

def _qsp_scalar(theta, phis):
    """Reference recurrence evaluated in float64 on a small grid."""
    theta = np.asarray(theta, dtype=np.float64)
    phis = np.asarray(phis, dtype=np.float64)
    c, s = np.cos(theta), np.sin(theta)
    r0r = np.ones_like(theta)
    r0i = np.zeros_like(theta)
    r1r = np.zeros_like(theta)
    r1i = np.zeros_like(theta)
    for phi in phis[1:]:
        cp, sp = np.cos(phi), np.sin(phi)
        ar = r0r * c - r1i * s
        ai = r0i * c + r1r * s
        br = r1r * c - r0i * s
        bi = r1i * c + r0r * s
        r0r = ar * cp - ai * sp
        r0i = ar * sp + ai * cp
        r1r = br * cp + bi * sp
        r1i = bi * cp - br * sp
    return r0r * np.cos(phis[0]) - r0i * np.sin(phis[0])


def _fourier_coeffs(phis):
    """f(theta) = c0 + sum A_k sin(2k theta + G_k); exact for the QSP chain."""
    j = np.arange(N_PHIS)
    theta_j = np.pi * j / N_PHIS
    C = np.fft.fft(_qsp_scalar(theta_j, phis)) / N_PHIS
    c0 = float(np.real(C[0]))
    A = 2.0 * np.abs(C[1 : QSP_DEPTH + 1])
    G = np.angle(C[1 : QSP_DEPTH + 1]) + np.pi / 2
    return c0, A, G


def _choose_T(c0, A):
    meanp = c0 * c0 + float((A**2).sum()) / 2.0  # mean f^2 over the period
    meanp = max(meanp, 1e-12)
    for T in range(T_MIN, T_MAX):
        tail = float((A[T:] ** 2).sum()) / 2.0
        if np.sqrt(tail / meanp) < REL_TOL_TARGET:
            return T
    return T_MAX


# ---------------------------------------------------------------------------
# device program
# ---------------------------------------------------------------------------

def _split_excess_waits(nc):
    """This walrus build only supports ONE sem-wait per instruction; spill
    extra waits onto preceding same-engine NOPs."""
    cnt = 0
    for bb in nc.main_func.blocks:
        out, changed = [], False
        for ins in list(bb.instructions):
            si = ins.sync_info
            if si is not None and len(si.on_wait) > 1:
                waits = list(si.on_wait)
                for w in waits[1:]:
                    nop = mb.InstNoOp(name=f"waitsplit-{cnt}", ins=[], outs=[])
                    cnt += 1
                    nop.engine = ins.engine
                    nop.sync_info = mb.SyncInfo(on_wait=[w], on_update=[])
                    nc.register_instruction(nop)
                    out.append(nop)
                ins.sync_info = mb.SyncInfo(on_wait=waits[:1], on_update=list(si.on_update))
                changed = True
            out.append(ins)
        if changed:
            bb.instructions = out
    return cnt


_NC_CACHE = {}


def _build_nc(T):
    if T in _NC_CACHE:
        return _NC_CACHE[T]

    nc = bass.Bass("TRN2", target_bir_lowering=False, debug=False, num_devices=N_CORES)
    f32, i32 = mb.dt.float32, mb.dt.int32

    th_d = nc.dram_tensor("theta", [P, F], f32, kind="ExternalInput")
    al_d = nc.dram_tensor("alphas", [P, F], f32, kind="ExternalInput")
    par_d = nc.dram_tensor("params", [P, 3 * T + 2], f32, kind="ExternalInput")
    dg_d = nc.dram_tensor("diags", [P, T * P], mb.dt.bfloat16, kind="ExternalInput")
    out_d = nc.dram_tensor("preds", [P, F], f32, kind="ExternalOutput")

    with tile.TileContext(nc) as tc:
        with (
            tc.tile_pool(name="persist", bufs=1) as persist,
            tc.tile_pool(name="work", bufs=1) as work,
            tc.tile_pool(name="psum", bufs=1, space="PSUM") as psum,
        ):
            th = persist.tile([P, F], f32, tag="th")
            nc.sync.dma_start(th[:], th_d.ap())
            al = persist.tile([P, F], f32, tag="al")
            nc.sync.dma_start(al[:], al_d.ap())
            par = persist.tile([P, 3 * T + 2], f32, tag="par")
            nc.sync.dma_start(par[:], par_d.ap())
            dg = persist.tile([P, T * P], mb.dt.bfloat16, tag="dg")
            nc.sync.dma_start(dg[:], dg_d.ap())

            acc = psum.tile([P, F], f32, tag="acc")

            for i in range(T):
                k = i + 1
                m_k = float(np.float32(np.pi / k))
                n_t = work.tile([P, F], mb.dt.int16, tag=f"n{i}")
                # n = int32((theta + s1) * (1/m))  -- trunc toward zero
                nc.vector.tensor_scalar(
                    n_t[:], th[:], par[:, 3 * i : 3 * i + 1], float(1.0 / (np.pi / k)),
                    mb.AluOpType.add, mb.AluOpType.mult,
                )
                r_t = work.tile([P, F], f32, tag=f"r{i}")
                eng = nc.gpsimd if (i % GPS_FIXUP_EVERY == GPS_FIXUP_EVERY - 1) else nc.vector
                eng.scalar_tensor_tensor(
                    r_t[:], n_t[:], -m_k, th[:],
                    mb.AluOpType.mult, mb.AluOpType.add,
                )
                h_t = work.tile([P, F], mb.dt.bfloat16, tag=f"h{i}")
                nc.scalar.activation(
                    h_t[:], r_t[:], mb.ActivationFunctionType.Sin,
                    bias=par[:, 3 * i + 1 : 3 * i + 2], scale=float(2.0 * k),
                )
                nc.tensor.matmul(
                    acc[:], dg[:, i * P : (i + 1) * P], h_t[:],
                    start=(i == 0), stop=(i == T - 1),
                )

            # preds = (acc + c0) * alphas + bias
            tmp = work.tile([P, F], f32, tag="tmp")
            nc.vector.scalar_tensor_tensor(
                tmp[:], acc[:], par[:, 3 * T : 3 * T + 1], al[:],
                mb.AluOpType.add, mb.AluOpType.mult,
            )
            pred_t = work.tile([P, F], f32, tag="pred")
            nc.vector.tensor_scalar(
                pred_t[:], tmp[:], par[:, 3 * T + 1 : 3 * T + 2], None,
                mb.AluOpType.add,
            )
            nc.sync.dma_start(out_d.ap(), pred_t[:])

    _split_excess_waits(nc)
    _NC_CACHE[T] = nc
    return nc


# ---------------------------------------------------------------------------
# entry point
# ---------------------------------------------------------------------------

def kernel(x, qsp_params, alphas, bias):
    x = np.asarray(x, dtype=np.float32)
    qsp_params = np.asarray(qsp_params, dtype=np.float64)
    alphas = np.asarray(alphas, dtype=np.float32)
    bias_v = float(np.asarray(bias, dtype=np.float64).reshape(-1)[0])

    c0, A, G = _fourier_coeffs(qsp_params)
    T = _choose_T(c0, A)

    ks = np.arange(1, T + 1, dtype=np.float64)
    m = np.pi / ks
    D = np.ceil(8.0 / m) * m  # positive shift, integer multiple of m
    delta = np.mod(G[:T], 2 * np.pi) / (2 * ks)  # in [0, m)
    s1 = delta + D  # ts1 add-scalar (HW fp32->int32 cast rounds to nearest)
    Bact = 2 * ks * (delta + D)  # ACT bias: arg = 2k*r + Bact in (-pi, pi]

    params = np.zeros((P, 3 * T + 2), dtype=np.float32)
    for i in range(T):
        params[:, 3 * i] = s1[i]
        params[:, 3 * i + 1] = Bact[i]
        params[:, 3 * i + 2] = A[i]  # unused by device now (baked in diags)
    params[:, 3 * T] = c0
    params[:, 3 * T + 1] = bias_v

    eye = np.eye(P, dtype=np.float32)
    import ml_dtypes
    diags = np.concatenate([A[i] * eye for i in range(T)], axis=1).astype(ml_dtypes.bfloat16)
    diags = np.ascontiguousarray(diags)

    theta = x[:, 0]
    per_core = B // N_CORES
    in_maps = []
    for c in range(N_CORES):
        sl = slice(c * per_core, (c + 1) * per_core)
        in_maps.append(
            {
                "theta": np.ascontiguousarray(theta[sl].reshape(P, F)),
                "alphas": np.ascontiguousarray(alphas[sl].reshape(P, F)),
                "params": params,
                "diags": diags,
            }
        )

    nc = _build_nc(T)
    res = run_bass_kernel_spmd(nc, in_maps, core_ids=list(range(N_CORES)))

    out = np.empty((B,), dtype=np.float32)
    for c in range(N_CORES):
        out[c * per_core : (c + 1) * per_core] = res.results[c]["preds"].reshape(-1)
    return out[:, None]


# revision 7
# speedup vs baseline: 1.1373x; 1.0003x over previous
"""QSP expectation kernel for Trainium2 (8 NeuronCores, data-parallel).

Math: the reference computes preds = alphas * Re(<0|U|0>) + bias where U is a
QSP chain with 55 phases. Re(<0|U|0>) as a function of theta is EXACTLY a
truncated Fourier series with period pi:

    f(theta) = c0 + sum_{k=1..27} A_k * sin(2k*theta + G_k)

(55 real degrees of freedom = 55 phases). The coefficients are recovered on
the host from 55 samples of the (cheap, 55-point) recurrence via FFT; the
spectrum decays exponentially in k, so only the first T (~14-20) harmonics
are needed for ~1e-3 relative accuracy.

Device kernel per core (65536 elements as [128, 512] f32), per harmonic k:
  1. DVE  tensor_scalar:  n = int32((theta + s1_k) * (1/m_k))   [trunc]
     with m_k = pi/k, s1_k = delta_k + D_k  (range reduction mod m_k; cast=RNE)
  2. DVE/GPSIMD stt:      r = (n * -m_k) + theta                 [residual]
  3. ACT  Sin:            h = sin(r * 2k + B_k),  arg in (-pi, pi]
  4. PE   diag-matmul:    acc(PSUM) += A_k * h    (lhsT = A_k * I, host-baked)
Epilogue: preds = (acc + c0) * alphas + bias  (DVE), DMA out.
"""

import numpy as np

import concourse.bass as bass
import concourse.tile as tile
from concourse import mybir as mb
import concourse.bass_utils as _bu
from concourse.bass_utils import run_bass_kernel_spmd

# The walrus NEFF epilogue clears its ENTIRE semaphore allocation space
# (~253 one-at-a-time EVENT_SEMAPHORE resets across engines = ~6us of tail).
# Our program uses ~10 semaphores; cap the allocator so the epilogue shrinks.
if not getattr(_bu, "_max_sem_patched", False):
    _orig_get_walrus_args = _bu.get_walrus_args

    def _patched_get_walrus_args(*a, **kw):
        return ["--max-sem-num=32", *_orig_get_walrus_args(*a, **kw)]

    _bu.get_walrus_args = _patched_get_walrus_args
    _bu._max_sem_patched = True

QSP_DEPTH = 27
N_PHIS = 2 * QSP_DEPTH + 1  # 55
B = 524288
N_CORES = 8
P = 128
F = B // N_CORES // P  # 512
REL_TOL_TARGET = 3e-3  # truncation budget (harness gate is 2e-2; bf16 adds ~1.5e-3)
T_MIN, T_MAX = 8, 27
GPS_FIXUP_EVERY = 10**9  # TensorScalarPtr is rejected on Pool by this walrus; keep fixups on DVE


# ---------------------------------------------------------------------------
# host-side math
# ---------------------------------------------------------------------------

def _qsp_scalar(theta, phis):
    """Reference recurrence evaluated in float64 on a small grid."""
    theta = np.asarray(theta, dtype=np.float64)
    phis = np.asarray(phis, dtype=np.float64)
    c, s = np.cos(theta), np.sin(theta)
    r0r = np.ones_like(theta)
    r0i = np.zeros_like(theta)
    r1r = np.zeros_like(theta)
    r1i = np.zeros_like(theta)
    for phi in phis[1:]:
        cp, sp = np.cos(phi), np.sin(phi)
        ar = r0r * c - r1i * s
        ai = r0i * c + r1r * s
        br = r1r * c - r0i * s
        bi = r1i * c + r0r * s
        r0r = ar * cp - ai * sp
        r0i = ar * sp + ai * cp
        r1r = br * cp + bi * sp
        r1i = bi * cp - br * sp
    return r0r * np.cos(phis[0]) - r0i * np.sin(phis[0])


def _fourier_coeffs(phis):
    """f(theta) = c0 + sum A_k sin(2k theta + G_k); exact for the QSP chain."""
    j = np.arange(N_PHIS)
    theta_j = np.pi * j / N_PHIS
    C = np.fft.fft(_qsp_scalar(theta_j, phis)) / N_PHIS
    c0 = float(np.real(C[0]))
    A = 2.0 * np.abs(C[1 : QSP_DEPTH + 1])
    G = np.angle(C[1 : QSP_DEPTH + 1]) + np.pi / 2
    return c0, A, G


def _choose_T(c0, A):
    meanp = c0 * c0 + float((A**2).sum()) / 2.0  # mean f^2 over the period
    meanp = max(meanp, 1e-12)
    for T in range(T_MIN, T_MAX):
        tail = float((A[T:] ** 2).sum()) / 2.0
        if np.sqrt(tail / meanp) < REL_TOL_TARGET:
            return T
    return T_MAX


# ---------------------------------------------------------------------------
# device program
# ---------------------------------------------------------------------------

def _split_excess_waits(nc):
    """This walrus build only supports ONE sem-wait per instruction; spill
    extra waits onto preceding same-engine NOPs."""
    cnt = 0
    for bb in nc.main_func.blocks:
        out, changed = [], False
        for ins in list(bb.instructions):
            si = ins.sync_info
            if si is not None and len(si.on_wait) > 1:
                waits = list(si.on_wait)
                for w in waits[1:]:
                    nop = mb.InstNoOp(name=f"waitsplit-{cnt}", ins=[], outs=[])
                    cnt += 1
                    nop.engine = ins.engine
                    nop.sync_info = mb.SyncInfo(on_wait=[w], on_update=[])
                    nc.register_instruction(nop)
                    out.append(nop)
                ins.sync_info = mb.SyncInfo(on_wait=waits[:1], on_update=list(si.on_update))
                changed = True
            out.append(ins)
        if changed:
            bb.instructions = out
    return cnt


_NC_CACHE = {}


def _build_nc(T):
    if T in _NC_CACHE:
        return _NC_CACHE[T]

    nc = bass.Bass("TRN2", target_bir_lowering=False, debug=False, num_devices=N_CORES)
    f32, i32 = mb.dt.float32, mb.dt.int32

    th_d = nc.dram_tensor("theta", [P, F], f32, kind="ExternalInput")
    al_d = nc.dram_tensor("alphas", [P, F], f32, kind="ExternalInput")
    par_d = nc.dram_tensor("params", [P, 3 * T + 2], f32, kind="ExternalInput")
    dg_d = nc.dram_tensor("diags", [P, T * P], mb.dt.bfloat16, kind="ExternalInput")
    out_d = nc.dram_tensor("preds", [P, F], f32, kind="ExternalOutput")

    with tile.TileContext(nc) as tc:
        with (
            tc.tile_pool(name="persist", bufs=1) as persist,
            tc.tile_pool(name="work", bufs=1) as work,
            tc.tile_pool(name="psum", bufs=1, space="PSUM") as psum,
        ):
            th = persist.tile([P, F], f32, tag="th")
            nc.sync.dma_start(th[:], th_d.ap())
            al = persist.tile([P, F], f32, tag="al")
            nc.sync.dma_start(al[:], al_d.ap())
            par = persist.tile([P, 3 * T + 2], f32, tag="par")
            nc.sync.dma_start(par[:], par_d.ap())
            dg = persist.tile([P, T * P], mb.dt.bfloat16, tag="dg")
            nc.sync.dma_start(dg[:], dg_d.ap())

            acc = psum.tile([P, F], f32, tag="acc")

            for i in range(T):
                k = i + 1
                m_k = float(np.float32(np.pi / k))
                n_t = work.tile([P, F], mb.dt.int16, tag=f"n{i}")
                # n = int32((theta + s1) * (1/m))  -- trunc toward zero
                nc.vector.tensor_scalar(
                    n_t[:], th[:], par[:, 3 * i : 3 * i + 1], float(1.0 / (np.pi / k)),
                    mb.AluOpType.add, mb.AluOpType.mult,
                )
                r_t = work.tile([P, F], f32, tag=f"r{i}")
                eng = nc.gpsimd if (i % GPS_FIXUP_EVERY == GPS_FIXUP_EVERY - 1) else nc.vector
                eng.scalar_tensor_tensor(
                    r_t[:], n_t[:], -m_k, th[:],
                    mb.AluOpType.mult, mb.AluOpType.add,
                )
                h_t = work.tile([P, F], mb.dt.bfloat16, tag=f"h{i}")
                nc.scalar.activation(
                    h_t[:], r_t[:], mb.ActivationFunctionType.Sin,
                    bias=par[:, 3 * i + 1 : 3 * i + 2], scale=float(2.0 * k),
                )
                nc.tensor.matmul(
                    acc[:], dg[:, i * P : (i + 1) * P], h_t[:],
                    start=(i == 0), stop=(i == T - 1),
                )

            # preds = (acc + c0) * alphas + bias
            tmp = work.tile([P, F], f32, tag="tmp")
            nc.vector.scalar_tensor_tensor(
                tmp[:], acc[:], par[:, 3 * T : 3 * T + 1], al[:],
                mb.AluOpType.add, mb.AluOpType.mult,
            )
            pred_t = work.tile([P, F], f32, tag="pred")
            nc.vector.tensor_scalar(
                pred_t[:], tmp[:], par[:, 3 * T + 1 : 3 * T + 2], None,
                mb.AluOpType.add,
            )
            nc.sync.dma_start(out_d.ap(), pred_t[:])

    _split_excess_waits(nc)
    _NC_CACHE[T] = nc
    return nc


# ---------------------------------------------------------------------------
# entry point
# ---------------------------------------------------------------------------

def kernel(x, qsp_params, alphas, bias):
    x = np.asarray(x, dtype=np.float32)
    qsp_params = np.asarray(qsp_params, dtype=np.float64)
    alphas = np.asarray(alphas, dtype=np.float32)
    bias_v = float(np.asarray(bias, dtype=np.float64).reshape(-1)[0])

    c0, A, G = _fourier_coeffs(qsp_params)
    T = _choose_T(c0, A)

    ks = np.arange(1, T + 1, dtype=np.float64)
    m = np.pi / ks
    D = np.ceil(8.0 / m) * m  # positive shift, integer multiple of m
    delta = np.mod(G[:T], 2 * np.pi) / (2 * ks)  # in [0, m)
    s1 = delta + D  # ts1 add-scalar (HW fp32->int32 cast rounds to nearest)
    Bact = 2 * ks * (delta + D)  # ACT bias: arg = 2k*r + Bact in (-pi, pi]

    params = np.zeros((P, 3 * T + 2), dtype=np.float32)
    for i in range(T):
        params[:, 3 * i] = s1[i]
        params[:, 3 * i + 1] = Bact[i]
        params[:, 3 * i + 2] = A[i]  # unused by device now (baked in diags)
    params[:, 3 * T] = c0
    params[:, 3 * T + 1] = bias_v

    eye = np.eye(P, dtype=np.float32)
    import ml_dtypes
    diags = np.concatenate([A[i] * eye for i in range(T)], axis=1).astype(ml_dtypes.bfloat16)
    diags = np.ascontiguousarray(diags)

    theta = x[:, 0]
    per_core = B // N_CORES
    in_maps = []
    for c in range(N_CORES):
        sl = slice(c * per_core, (c + 1) * per_core)
        in_maps.append(
            {
                "theta": np.ascontiguousarray(theta[sl].reshape(P, F)),
                "alphas": np.ascontiguousarray(alphas[sl].reshape(P, F)),
                "params": params,
                "diags": diags,
            }
        )

    nc = _build_nc(T)
    res = run_bass_kernel_spmd(nc, in_maps, core_ids=list(range(N_CORES)))

    out = np.empty((B,), dtype=np.float32)
    for c in range(N_CORES):
        out[c * per_core : (c + 1) * per_core] = res.results[c]["preds"].reshape(-1)
    return out[:, None]
